# revision 21
# baseline (speedup 1.0000x reference)
"""Trainium2 Bass kernel: per-sample position-decay mask multiply.

out[b, l, h] = data[b, l, h] * mask[b, l]
  mask[b, l] = 1 - (a_end - l)/C           if l < a_end
             = 1 - (l - a_idx)/C           elif l < sents_len
             = 0                           otherwise
  with a_end = aspect_Index + aspect_len, C = 40.

Sharding: data-parallel over the batch (64 samples per core), plus a ragged
skip: for l >= act = max(a_end, sents_len) the output is structurally zero,
and kernel outputs are pre-zeroed, so those regions need no traffic at all.

Each sample is split into T_SEG segments of LTT = L/T_SEG positions. A
(sample, seg) row is active iff act > seg*LTT. The host sorts each core's
samples by act descending and packs rows seg-block by seg-block, so active
rows form a dense prefix [0, R) — plain rectangular DMAs, no indirection.
Padded rows (sample with act <= seg start) compute an all-zero mask and
write zeros, which is exactly their correct output. R is baked into the
compiled program per invocation (compile is cached by R).

On device: rows stream through SBUF in [<=128, W] tiles; a per-row position
mask ([rows, LTT]) is computed once from an iota and 4 per-row scalars
(position offsets folded into the scalars host-side; all values are small
integers, exact in f32), then broadcast-multiplied into the stream.
"""

import numpy as np

import concourse.bacc as bacc
import concourse.mybir as mybir
import concourse.tile as tile
from concourse.bass_utils import run_bass_kernel_spmd

N_CORES = 8
B, L, H = 512, 512, 100
BS = B // N_CORES          # 64 samples per core
T_SEG = 2                  # segments per sample (ragged granularity)
LTT = L // T_SEG           # positions per segment
XT = LTT * H               # f32 elements per row
C = 40.0
W = 3200                   # main-loop tile width (f32 elems per row)
LW = W // H                # positions per tile
NT = XT // W               # tiles per row-group
PMAX = 128                 # SBUF partitions per row-group

F32 = mybir.dt.float32


def build_bass(R):
    """Build + compile the SPMD program for R packed rows per core."""
    nc = bacc.Bacc("TRN2", target_bir_lowering=False, debug=False)

    data = nc.dram_tensor("data", [R, XT], F32, kind="ExternalInput")
    out = nc.dram_tensor("out", [R, XT], F32, kind="ExternalOutput")
    # Per-row scalars (host precomputed, seg offset absorbed; see module doc):
    #   0: a_end - C - off, 1: a_idx + C - off, 2: a_end - off, 3: slen - off
    scals = nc.dram_tensor("scals", [R, 4], F32, kind="ExternalInput")

    groups = [(g * PMAX, min(PMAX, R - g * PMAX))
              for g in range((R + PMAX - 1) // PMAX)]

    with tile.TileContext(nc) as tc:
        with (
            tc.tile_pool(name="consts", bufs=1) as consts,
            tc.tile_pool(name="io", bufs=4) as io,
        ):
            # iota over local positions j = 0..LTT-1, same in every row
            iota_i = consts.tile([PMAX, LTT], mybir.dt.int32, tag="iota_i")
            nc.gpsimd.iota(iota_i[:], pattern=[[1, LTT]], base=0,
                           channel_multiplier=0)
            # Funnel: single DVE copy waits on gpsimd; everything after is
            # same-engine (DVE) ordered, so each op needs <=1 sem wait.
            iota_f = consts.tile([PMAX, LTT], F32, tag="iota_f")
            nc.vector.tensor_copy(iota_f[:], iota_i[:])

            # per-group mask: where(j < aend', (j - aec')/C,
            #                       where(j < slen', -(j - aic')/C, 0))
            masks = []
            for gi, (r0, rows) in enumerate(groups):
                scal_t = consts.tile([PMAX, 4], F32, tag=f"scals{gi}")
                nc.sync.dma_start(scal_t[:rows, :], scals.ap()[r0:r0 + rows, :])

                mask_t = consts.tile([PMAX, LTT], F32, tag=f"mask{gi}")
                t1 = consts.tile([PMAX, LTT], F32, tag="t1")
                c2 = consts.tile([PMAX, LTT], F32, tag="c2")
                c1 = consts.tile([PMAX, LTT], mybir.dt.uint8, tag="c1")

                def col(k, rows=rows, scal_t=scal_t):
                    return scal_t[:rows, k:k + 1].broadcast_to([rows, LTT])

                io_f = iota_f[:rows, :]
                nc.vector.tensor_tensor(out=t1[:rows, :], in0=io_f, in1=col(0),
                                        op=mybir.AluOpType.subtract)
                nc.vector.tensor_scalar(
                    out=t1[:rows, :], in0=t1[:rows, :], scalar1=1.0 / C,
                    scalar2=None, op0=mybir.AluOpType.mult)
                nc.vector.tensor_tensor(out=mask_t[:rows, :], in0=io_f,
                                        in1=col(1), op=mybir.AluOpType.subtract)
                nc.vector.tensor_scalar(
                    out=mask_t[:rows, :], in0=mask_t[:rows, :], scalar1=-1.0 / C,
                    scalar2=None, op0=mybir.AluOpType.mult)
                nc.vector.tensor_tensor(out=c2[:rows, :], in0=io_f, in1=col(3),
                                        op=mybir.AluOpType.is_lt)
                nc.vector.tensor_tensor(out=mask_t[:rows, :],
                                        in0=mask_t[:rows, :], in1=c2[:rows, :],
                                        op=mybir.AluOpType.mult)
                nc.vector.tensor_tensor(out=c1[:rows, :], in0=io_f, in1=col(2),
                                        op=mybir.AluOpType.is_lt)
                nc.vector.copy_predicated(mask_t[:rows, :], c1[:rows, :],
                                          t1[:rows, :])
                masks.append(mask_t)

            for i in range(NT):
                for gi, (r0, rows) in enumerate(groups):
                    t = io.tile([PMAX, W], F32, tag="io")
                    # loads on the SP HWDGE ring, stores on the ACT ring —
                    # the two FIFOs issue concurrently
                    nc.sync.dma_start(
                        t[:rows, :], data.ap()[r0:r0 + rows, i * W:(i + 1) * W])
                    d3 = t[:rows, :].rearrange("p (l h) -> p l h", h=H)
                    m3 = masks[gi][:rows, i * LW:(i + 1) * LW].unsqueeze(
                        2).broadcast_to([rows, LW, H])
                    nc.vector.tensor_tensor(out=d3, in0=d3, in1=m3,
                                            op=mybir.AluOpType.mult)
                    nc.scalar.dma_start(
                        out.ap()[r0:r0 + rows, i * W:(i + 1) * W], t[:rows, :])

    nc.compile()
    return nc


_NC_CACHE = {}


def _get_nc(R):
    if R not in _NC_CACHE:
        _NC_CACHE[R] = build_bass(R)
    return _NC_CACHE[R]


def plan_and_pack(data, aspect_Index, aspect_len, sents_len):
    """Shard samples across cores (balanced by active length), pack active
    (sample, seg) rows into dense per-core buffers, build per-row scalars."""
    data = np.asarray(data, dtype=np.float32)
    a_idx = np.asarray(aspect_Index).astype(np.int64)
    a_end = a_idx + np.asarray(aspect_len).astype(np.int64)
    s_len = np.asarray(sents_len).astype(np.int64)
    act = np.maximum(a_end, s_len)

    # deal samples round-robin from the act-descending order: equalizes the
    # per-core count of rows above every threshold to +-1
    order = np.argsort(-act, kind="stable")
    cores = [order[c::N_CORES] for c in range(N_CORES)]  # each desc in act

    # K[s] = max over cores of #samples with act > s*LTT  (same for all
    # cores after padding; padded rows produce zeros, which is correct)
    K = [max(int((act[m] > s * LTT).sum()) for m in cores) or (1 if s == 0 else 0)
         for s in range(T_SEG)]
    K = [k for k in K if k > 0]
    R = sum(K)

    data3 = data.reshape(B, T_SEG, XT)
    in_maps, recon = [], []
    for c in range(N_CORES):
        mine = cores[c]
        rows_sample = np.concatenate([mine[:k] for k in K])          # [R]
        rows_seg = np.concatenate([np.full(k, s) for s, k in enumerate(K)])
        buf = np.ascontiguousarray(data3[rows_sample, rows_seg, :])  # [R, XT]

        offv = rows_seg.astype(np.float64) * LTT
        aend_v = a_end[rows_sample].astype(np.float64) - offv
        aidx_v = a_idx[rows_sample].astype(np.float64) - offv
        slen_v = s_len[rows_sample].astype(np.float64) - offv
        scal = np.stack([aend_v - C, aidx_v + C, aend_v, slen_v],
                        axis=1).astype(np.float32)
        in_maps.append({"data": buf, "scals": np.ascontiguousarray(scal)})
        recon.append((rows_sample, rows_seg))
    return in_maps, recon, R


def kernel(data, aspect_Index, aspect_len, sents_len):
    in_maps, recon, R = plan_and_pack(data, aspect_Index, aspect_len, sents_len)
    nc = _get_nc(R)
    res = run_bass_kernel_spmd(nc, in_maps, list(range(N_CORES)))
    out = np.zeros((B, T_SEG, XT), dtype=np.float32)
    for c in range(N_CORES):
        rows_sample, rows_seg = recon[c]
        out[rows_sample, rows_seg, :] = res.results[c]["out"]
    return out.reshape(B, L, H)


if __name__ == "__main__":
    rng = np.random.default_rng(1)
    d = rng.standard_normal((B, L, H), dtype=np.float32)
    ai = rng.integers(0, 100, B).astype(np.int64)
    al = rng.integers(0, 10, B).astype(np.int64)
    slv = rng.integers(0, 512, B).astype(np.int64)
    got = kernel(d, ai, al, slv)
    i = np.arange(L, dtype=np.float32)[None, :]
    ae = (ai + al).astype(np.float32)[:, None]
    aif = ai.astype(np.float32)[:, None]
    m = np.where(i < ae, 1.0 - (ae - i) / C,
                 np.where(i < slv[:, None], 1.0 - (i - aif) / C, 0.0))
    want = d * m[:, :, None].astype(np.float32)
    print("selftest max abs err:", np.abs(got - want).max())


# revision 24
# speedup vs baseline: 12.9180x; 12.9180x over previous
"""Trainium2 Bass kernel: per-sample position-decay mask multiply.

out[b, l, h] = data[b, l, h] * mask[b, l]
  mask[b, l] = 1 - (a_end - l)/C           if l < a_end
             = 1 - (l - a_idx)/C           elif l < sents_len
             = 0                           otherwise
  with a_end = aspect_Index + aspect_len, C = 40.

Sharding: data-parallel over the batch (64 samples per core), plus a ragged
skip: for l >= act = max(a_end, sents_len) the output is structurally zero,
and kernel outputs are pre-zeroed, so those regions need no traffic at all.

Each sample is split into T_SEG segments of LTT = L/T_SEG positions. A
(sample, seg) row is active iff act > seg*LTT. The host sorts each core's
samples by act descending and packs rows seg-block by seg-block, so active
rows form a dense prefix [0, R) — plain rectangular DMAs, no indirection.
Padded rows (sample with act <= seg start) compute an all-zero mask and
write zeros, which is exactly their correct output. R is baked into the
compiled program per invocation (compile is cached by R).

On device: rows stream through SBUF in [<=128, W] tiles; a per-row position
mask ([rows, LTT]) is computed once from an iota and 4 per-row scalars
(position offsets folded into the scalars host-side; all values are small
integers, exact in f32), then broadcast-multiplied into the stream.
"""

import numpy as np

import concourse.bacc as bacc
import concourse.mybir as mybir
import concourse.tile as tile
from concourse.bass_utils import run_bass_kernel_spmd

N_CORES = 8
B, L, H = 512, 512, 100
BS = B // N_CORES          # 64 samples per core
T_SEG = 16                 # segments per sample (ragged granularity)
LTT = L // T_SEG           # positions per segment
XT = LTT * H               # f32 elements per row
C = 40.0
W = XT                     # main-loop tile width (f32 elems per row)
LW = W // H                # positions per tile
NT = XT // W               # tiles per row-group
PMAX = 128                 # SBUF partitions per row-group

F32 = mybir.dt.float32


def build_bass(R):
    """Build + compile the SPMD program for R packed rows per core."""
    nc = bacc.Bacc("TRN2", target_bir_lowering=False, debug=False)

    data = nc.dram_tensor("data", [R, XT], F32, kind="ExternalInput")
    out = nc.dram_tensor("out", [R, XT], F32, kind="ExternalOutput")
    # Per-row scalars (host precomputed, seg offset absorbed; see module doc):
    #   0: a_end - C - off, 1: a_idx + C - off, 2: a_end - off, 3: slen - off
    scals = nc.dram_tensor("scals", [R, 4], F32, kind="ExternalInput")

    groups = [(g * PMAX, min(PMAX, R - g * PMAX))
              for g in range((R + PMAX - 1) // PMAX)]

    with tile.TileContext(nc) as tc:
        with (
            tc.tile_pool(name="consts", bufs=1) as consts,
            tc.tile_pool(name="io", bufs=4) as io,
        ):
            # iota over local positions j = 0..LTT-1, same in every row
            iota_i = consts.tile([PMAX, LTT], mybir.dt.int32, tag="iota_i")
            nc.gpsimd.iota(iota_i[:], pattern=[[1, LTT]], base=0,
                           channel_multiplier=0)
            # Funnel: single DVE copy waits on gpsimd; everything after is
            # same-engine (DVE) ordered, so each op needs <=1 sem wait.
            iota_f = consts.tile([PMAX, LTT], F32, tag="iota_f")
            nc.vector.tensor_copy(iota_f[:], iota_i[:])

            # per-group mask: where(j < aend', (j - aec')/C,
            #                       where(j < slen', -(j - aic')/C, 0))
            masks = []
            for gi, (r0, rows) in enumerate(groups):
                scal_t = consts.tile([PMAX, 4], F32, tag=f"scals{gi}")
                nc.sync.dma_start(scal_t[:rows, :], scals.ap()[r0:r0 + rows, :])

                mask_t = consts.tile([PMAX, LTT], F32, tag=f"mask{gi}")
                t1 = consts.tile([PMAX, LTT], F32, tag="t1")
                c2 = consts.tile([PMAX, LTT], F32, tag="c2")
                c1 = consts.tile([PMAX, LTT], mybir.dt.uint8, tag="c1")

                def col(k, rows=rows, scal_t=scal_t):
                    return scal_t[:rows, k:k + 1].broadcast_to([rows, LTT])

                io_f = iota_f[:rows, :]
                nc.vector.tensor_tensor(out=t1[:rows, :], in0=io_f, in1=col(0),
                                        op=mybir.AluOpType.subtract)
                nc.vector.tensor_scalar(
                    out=t1[:rows, :], in0=t1[:rows, :], scalar1=1.0 / C,
                    scalar2=None, op0=mybir.AluOpType.mult)
                nc.vector.tensor_tensor(out=mask_t[:rows, :], in0=io_f,
                                        in1=col(1), op=mybir.AluOpType.subtract)
                nc.vector.tensor_scalar(
                    out=mask_t[:rows, :], in0=mask_t[:rows, :], scalar1=-1.0 / C,
                    scalar2=None, op0=mybir.AluOpType.mult)
                nc.vector.tensor_tensor(out=c2[:rows, :], in0=io_f, in1=col(3),
                                        op=mybir.AluOpType.is_lt)
                nc.vector.tensor_tensor(out=mask_t[:rows, :],
                                        in0=mask_t[:rows, :], in1=c2[:rows, :],
                                        op=mybir.AluOpType.mult)
                nc.vector.tensor_tensor(out=c1[:rows, :], in0=io_f, in1=col(2),
                                        op=mybir.AluOpType.is_lt)
                nc.vector.copy_predicated(mask_t[:rows, :], c1[:rows, :],
                                          t1[:rows, :])
                masks.append(mask_t)

            for i in range(NT):
                for gi, (r0, rows) in enumerate(groups):
                    t = io.tile([PMAX, W], F32, tag="io")
                    # loads on the SP HWDGE ring, stores on the ACT ring —
                    # the two FIFOs issue concurrently
                    nc.sync.dma_start(
                        t[:rows, :], data.ap()[r0:r0 + rows, i * W:(i + 1) * W])
                    d3 = t[:rows, :].rearrange("p (l h) -> p l h", h=H)
                    m3 = masks[gi][:rows, i * LW:(i + 1) * LW].unsqueeze(
                        2).broadcast_to([rows, LW, H])
                    nc.vector.tensor_tensor(out=d3, in0=d3, in1=m3,
                                            op=mybir.AluOpType.mult)
                    nc.scalar.dma_start(
                        out.ap()[r0:r0 + rows, i * W:(i + 1) * W], t[:rows, :])

    nc.compile()
    return nc


_NC_CACHE = {}


def _get_nc(R):
    if R not in _NC_CACHE:
        _NC_CACHE[R] = build_bass(R)
    return _NC_CACHE[R]


def plan_and_pack(data, aspect_Index, aspect_len, sents_len):
    """Shard samples across cores (balanced by active length), pack active
    (sample, seg) rows into dense per-core buffers, build per-row scalars."""
    data = np.asarray(data, dtype=np.float32)
    a_idx = np.asarray(aspect_Index).astype(np.int64)
    a_end = a_idx + np.asarray(aspect_len).astype(np.int64)
    s_len = np.asarray(sents_len).astype(np.int64)
    act = np.maximum(a_end, s_len)

    # deal samples round-robin from the act-descending order: equalizes the
    # per-core count of rows above every threshold to +-1
    order = np.argsort(-act, kind="stable")
    cores = [order[c::N_CORES] for c in range(N_CORES)]  # each desc in act

    # K[s] = max over cores of #samples with act > s*LTT  (same for all
    # cores after padding; padded rows produce zeros, which is correct)
    K = [max(int((act[m] > s * LTT).sum()) for m in cores) or (1 if s == 0 else 0)
         for s in range(T_SEG)]
    K = [k for k in K if k > 0]
    R = sum(K)
    # DMAs only reach full SDMA-engine spread at exactly 128 partitions, so
    # pad the row count to a multiple of 128 with dummy all-zero-mask rows.
    RP = -(-R // 128) * 128

    data3 = data.reshape(B, T_SEG, XT)
    in_maps, recon = [], []
    for c in range(N_CORES):
        mine = cores[c]
        rows_sample = np.concatenate([mine[:k] for k in K])          # [R]
        rows_seg = np.concatenate([np.full(k, s) for s, k in enumerate(K)])
        buf = np.zeros((RP, XT), dtype=np.float32)
        buf[:R] = data3[rows_sample, rows_seg, :]

        offv = rows_seg.astype(np.float64) * LTT
        aend_v = a_end[rows_sample].astype(np.float64) - offv
        aidx_v = a_idx[rows_sample].astype(np.float64) - offv
        slen_v = s_len[rows_sample].astype(np.float64) - offv
        scal = np.full((RP, 4), -1e6, dtype=np.float32)  # dummy: mask == 0
        scal[:R] = np.stack([aend_v - C, aidx_v + C, aend_v, slen_v],
                            axis=1).astype(np.float32)
        in_maps.append({"data": buf, "scals": scal})
        recon.append((rows_sample, rows_seg))
    return in_maps, recon, RP


def kernel(data, aspect_Index, aspect_len, sents_len):
    in_maps, recon, R = plan_and_pack(data, aspect_Index, aspect_len, sents_len)
    nc = _get_nc(R)
    res = run_bass_kernel_spmd(nc, in_maps, list(range(N_CORES)))
    out = np.zeros((B, T_SEG, XT), dtype=np.float32)
    for c in range(N_CORES):
        rows_sample, rows_seg = recon[c]
        out[rows_sample, rows_seg, :] = res.results[c]["out"][:len(rows_sample)]
    return out.reshape(B, L, H)


if __name__ == "__main__":
    rng = np.random.default_rng(1)
    d = rng.standard_normal((B, L, H), dtype=np.float32)
    ai = rng.integers(0, 100, B).astype(np.int64)
    al = rng.integers(0, 10, B).astype(np.int64)
    slv = rng.integers(0, 512, B).astype(np.int64)
    got = kernel(d, ai, al, slv)
    i = np.arange(L, dtype=np.float32)[None, :]
    ae = (ai + al).astype(np.float32)[:, None]
    aif = ai.astype(np.float32)[:, None]
    m = np.where(i < ae, 1.0 - (ae - i) / C,
                 np.where(i < slv[:, None], 1.0 - (i - aif) / C, 0.0))
    want = d * m[:, :, None].astype(np.float32)
    print("selftest max abs err:", np.abs(got - want).max())
